# revision 3
# baseline (speedup 1.0000x reference)
"""Trainium2 Bass kernel for nn_CT_loss (data-parallel over batch, 8 cores).

Math (R is a general 3x3 matrix, not orthogonal):
  u   = A P0 + b0          A = R diag(e), b0 = t - 0.5 R e      (per batch)
  c   = G P0 + g0          G = R^T A,     g0 = R^T b0
  v_a = A[:,c1] Qa' + A[:,c2] Qb' + h_a  (Q' = Q-0.5), s = R^T t
  d_a = s_a u - c_a v_a ;  la = sqrt(|d_a|^2 m_a)
  loss = sum_a [sum(m_a) >= 3B] sum(la) / max(sum_a sum(m_a), 1)

Device trick: fold 1/s_a into v's affine coefficients (vt = v/s_a), so
  d~_a = u - c_a vt_a  is scalar-free; host multiplies the per-batch
  partial sums by |s_a| during the gather.

v2 design: 3-engine balance.
  - DVE: all tensor_tensor (2-src) work + c-chain via scalar_tensor_tensor
  - ACT: biased terms, tv1 terms, squares (wide), sqrt+accum
  - GPSIMD: tv2/tv1 one-src affine terms (tensor_scalar)
  - DMA order: q01 -> p0 -> q23 -> q45 -> mk so the critical v-chain
    starts earliest.

Layout per core: 8 batches; tiles [128, FD=1024]; partition = b*16+g,
free = 1024 pixels. Per-batch scalars ride as per-partition [128,1]
columns of a constants tile. Free-dim sums via accum_out; host finishes
the 128-row + cross-core reduction (the "gather").
"""
import os
import sys

import numpy as np

for _p in ("/opt/trn_rl_repo",):
    if _p not in sys.path:
        sys.path.insert(0, _p)

import concourse.bass as bass
import concourse.bacc as bacc
import concourse.tile as tile
from concourse import mybir
from concourse.bass_utils import run_bass_kernel_spmd

from ml_dtypes import bfloat16

F32 = mybir.dt.float32
BF16 = mybir.dt.bfloat16
AF = mybir.ActivationFunctionType
OP = mybir.AluOpType

B, HW = 64, 128 * 128
NCORES, BPC, G, FD = 8, 8, 16, 1024
F3 = 3 * FD

# a -> (Acol1, Acol2, qchA, qchB)
QCH = {0: (1, 2, 0, 1), 1: (0, 2, 2, 3), 2: (0, 1, 4, 5)}

# constants tile columns
CA = 0    # A[i*3+j] 9
CB0 = 9   # b0 3
CG = 12   # G[a*3+j] 9
CG0 = 21  # g0 3
CV1 = 24  # alpha~[a*3+i] = A[i,c1]/s~_a 9
CHC = 33  # h~[a*3+i] 9
CV2 = 42  # beta~[a*3+i] = A[i,c2]/s~_a 9
CZ = 51   # 0.0
NCST = 52

_BUILT = None
LAST = None


def _bcast3(ap, n):
    """[128, FD] AP -> [128, n, FD] with step-0 middle dim."""
    return bass.AP(tensor=ap.tensor, offset=ap.offset,
                   ap=[ap.ap[0], [0, n], *ap.ap[1:]])


def _build_nc():
    nc = bacc.Bacc(None)
    p0 = nc.dram_tensor("p0", [BPC, G, 3, FD], BF16, kind="ExternalInput")
    q0 = nc.dram_tensor("q0", [BPC, G, 6, FD], BF16, kind="ExternalInput")
    mk = nc.dram_tensor("mk", [BPC, G, 3, FD], BF16, kind="ExternalInput")
    cst = nc.dram_tensor("cst", [128, NCST], F32, kind="ExternalInput")
    outp = nc.dram_tensor("out", [128, 3], F32, kind="ExternalOutput")

    with tile.TileContext(nc) as tc:
        with tc.tile_pool(name="main", bufs=1) as pool:
            # constants first (tiny, scalar HWDGE ring)
            cst_t = pool.tile([128, NCST], F32, tag="cst")
            nc.scalar.dma_start(cst_t[:], cst[:])

            def cs(j):
                return cst_t[:, j:j + 1]

            zero = cs(CZ)

            # warm the ACT table set early (overlaps the input DMA)
            warm = pool.tile([128, 1], BF16, tag="warm")
            nc.scalar.activation(warm[:], cst_t[:, CZ:CZ + 1], AF.Sqrt)

            # ---- input tiles + DMA (sync ring), critical-path order ----
            p0r = p0[:].rearrange("b g c f -> (b g) c f")
            q0r = q0[:].rearrange("b g c f -> (b g) c f")
            mkr = mk[:].rearrange("b g c f -> (b g) c f")

            p0_t = pool.tile([128, 3, FD], BF16, tag="p0")
            q0_t = pool.tile([128, 6, FD], BF16, tag="q0")
            mk_t = pool.tile([128, 3, FD], BF16, tag="mk")

            nc.sync.dma_start(q0_t[:, 0:2, :], q0r[:, 0:2, :])   # a=0 pair
            nc.sync.dma_start(p0_t[:], p0r[:])
            nc.sync.dma_start(q0_t[:, 2:4, :], q0r[:, 2:4, :])   # a=1 pair
            nc.sync.dma_start(q0_t[:, 4:6, :], q0r[:, 4:6, :])   # a=2 pair
            nc.sync.dma_start(mk_t[:], mkr[:])

            acc = pool.tile([128, 3], F32, tag="acc")

            X = [p0_t[:, j, :] for j in range(3)]
            Q = [q0_t[:, j, :] for j in range(6)]
            MSK = [mk_t[:, a, :] for a in range(3)]

            # ---- tiles ----
            u3 = pool.tile([128, 3, FD], BF16, tag="u3")
            ut2 = pool.tile([128, 3, FD], BF16, tag="ut2")
            ut1 = pool.tile([128, 3, FD], BF16, tag="ut1")
            ut0 = pool.tile([128, 3, FD], BF16, tag="ut0")
            ct2 = pool.tile([128, 3, FD], BF16, tag="ct2")
            cs1 = pool.tile([128, 3, FD], BF16, tag="cs1")
            c3 = pool.tile([128, 3, FD], BF16, tag="c3")
            tv1 = [pool.tile([128, 3, FD], BF16, name=f"tv1_{a}", tag=f"tv1_{a}")
                   for a in range(3)]
            tv2 = [pool.tile([128, 3, FD], BF16, name=f"tv2_{a}", tag=f"tv2_{a}")
                   for a in range(3)]
            va = [pool.tile([128, 3, FD], BF16, name=f"va{a}", tag=f"va{a}")
                  for a in range(3)]
            sq = [pool.tile([128, 3, FD], BF16, name=f"sq{a}", tag=f"sq{a}")
                  for a in range(3)]
            wt = [pool.tile([128, FD], BF16, name=f"w{a}", tag=f"w{a}")
                  for a in range(3)]
            la = [pool.tile([128, FD], BF16, name=f"la{a}", tag=f"la{a}")
                  for a in range(3)]

            # ============ issue in rough dataflow order ============
            # GPSIMD: tv2 a=0 (q01 first), then tv1/tv2 for a=1,2
            def gp_term(out, src, scj, bij):
                nc.gpsimd.tensor_scalar(out, src, cs(scj), cs(bij),
                                        op0=OP.mult, op1=OP.add)

            def act_term(out, src, scj, bij):
                nc.scalar.activation(out, src, AF.Identity,
                                     bias=cs(bij), scale=cs(scj))

            # a=0 v-terms: tv1 on ACT (biased), tv2 on GP
            a = 0
            c1a, c2a, qA, qB = QCH[a]
            for i in range(3):
                gp_term(tv2[a][:, i, :], Q[qB], CV2 + 3 * a + i, CZ)
            for i in range(3):
                act_term(tv1[a][:, i, :], Q[qA], CV1 + 3 * a + i, CHC + 3 * a + i)

            # u terms: t2 (bias) + t1 on ACT, t0 on DVE TS
            for i in range(3):
                act_term(ut2[:, i, :], X[2], CA + 3 * i + 2, CB0 + i)
            for i in range(3):
                nc.vector.tensor_scalar(ut0[:, i, :], X[0], cs(CA + 3 * i),
                                        None, op0=OP.mult)
            for i in range(3):
                act_term(ut1[:, i, :], X[1], CA + 3 * i + 1, CZ)

            # c chain: t2 on GP (biased), then 2x stt on DVE
            for aa in range(3):
                gp_term(ct2[:, aa, :], X[2], CG + 3 * aa + 2, CG0 + aa)

            # v0 = tv1 + tv2 (DVE), u = t2+t1+t0 (DVE)
            nc.vector.tensor_add(va[0][:], tv1[0][:], tv2[0][:])
            nc.vector.tensor_add(u3[:], ut2[:], ut1[:])
            nc.vector.tensor_add(u3[:], u3[:], ut0[:])

            # c: cs1_a = p1*G_a1 + ct2_a ; c_a = p0*G_a0 + cs1_a  (stt probe)
            for aa in range(3):
                nc.vector.scalar_tensor_tensor(
                    cs1[:, aa, :], X[1], cs(CG + 3 * aa + 1), ct2[:, aa, :],
                    op0=OP.mult, op1=OP.add)
            for aa in range(3):
                nc.vector.scalar_tensor_tensor(
                    c3[:, aa, :], X[0], cs(CG + 3 * aa), cs1[:, aa, :],
                    op0=OP.mult, op1=OP.add)

            # GP: remaining v terms for a=1, a=2
            for a in (1, 2):
                c1a, c2a, qA, qB = QCH[a]
                for i in range(3):
                    gp_term(tv2[a][:, i, :], Q[qB], CV2 + 3 * a + i, CZ)
                for i in range(3):
                    gp_term(tv1[a][:, i, :], Q[qA], CV1 + 3 * a + i,
                            CHC + 3 * a + i)

            # a=0: cv, d, sq
            nc.vector.tensor_mul(va[0][:], _bcast3(c3[:, 0, :], 3), va[0][:])
            nc.vector.tensor_sub(va[0][:], u3[:], va[0][:])
            nc.scalar.activation(sq[0][:], va[0][:], AF.Square)

            # a=1: v-add, cv, d, sq
            nc.vector.tensor_add(va[1][:], tv1[1][:], tv2[1][:])
            nc.vector.tensor_mul(va[1][:], _bcast3(c3[:, 1, :], 3), va[1][:])
            nc.vector.tensor_sub(va[1][:], u3[:], va[1][:])
            nc.scalar.activation(sq[1][:], va[1][:], AF.Square)

            # a=2
            nc.vector.tensor_add(va[2][:], tv1[2][:], tv2[2][:])
            nc.vector.tensor_mul(va[2][:], _bcast3(c3[:, 2, :], 3), va[2][:])
            nc.vector.tensor_sub(va[2][:], u3[:], va[2][:])
            nc.scalar.activation(sq[2][:], va[2][:], AF.Square)

            # tails: w = sum sq, * mask, sqrt+accum
            for a in range(3):
                nc.vector.tensor_add(wt[a], sq[a][:, 0, :], sq[a][:, 1, :])
                nc.vector.tensor_add(wt[a], wt[a], sq[a][:, 2, :])
                nc.vector.tensor_mul(wt[a], wt[a], MSK[a])
                nc.scalar.activation(la[a], wt[a], AF.Sqrt,
                                     accum_out=acc[:, a:a + 1])

            nc.sync.dma_start(outp[:], acc[:])

    nc.compile()
    return nc


def get_nc():
    global _BUILT
    if _BUILT is None:
        _BUILT = _build_nc()
    return _BUILT


def host_constants(R, T, E):
    """[B, NCST] fp32 constants (fp64 host math) + [B,3] |s| scales."""
    Bn = R.shape[0]
    out = np.zeros((Bn, NCST), np.float64)
    sabs = np.zeros((Bn, 3), np.float64)
    for b in range(Bn):
        Rb = R[b].astype(np.float64)
        tb = T[b].astype(np.float64)
        eb = E[b].astype(np.float64)
        A = Rb * eb[None, :]
        b0 = tb - 0.5 * (Rb @ eb)
        Gm = Rb.T @ A
        g0 = Rb.T @ b0
        s = Rb.T @ tb
        out[b, CA:CA + 9] = A.reshape(-1)
        out[b, CB0:CB0 + 3] = b0
        out[b, CG:CG + 9] = Gm.reshape(-1)
        out[b, CG0:CG0 + 3] = g0
        for a, (c1, c2, _, _) in QCH.items():
            sh = np.sign(s[a]) * max(abs(s[a]), 1e-12) if s[a] != 0 else 1e-12
            sabs[b, a] = abs(s[a])
            h = tb - 0.5 * (A[:, c1] + A[:, c2])
            out[b, CV1 + 3 * a:CV1 + 3 * a + 3] = A[:, c1] / sh
            out[b, CV2 + 3 * a:CV2 + 3 * a + 3] = A[:, c2] / sh
            out[b, CHC + 3 * a:CHC + 3 * a + 3] = h / sh
    return out.astype(np.float32), sabs


def make_in_maps(P0, Q0, M, cst):
    in_maps = []
    for k in range(NCORES):
        sl = slice(k * BPC, (k + 1) * BPC)
        in_maps.append({
            "p0": P0[sl].reshape(BPC, 3, G, FD).transpose(0, 2, 1, 3).astype(bfloat16),
            "q0": Q0[sl].reshape(BPC, 6, G, FD).transpose(0, 2, 1, 3).astype(bfloat16),
            "mk": M[sl].reshape(BPC, 3, G, FD).transpose(0, 2, 1, 3).astype(bfloat16),
            "cst": np.ascontiguousarray(np.repeat(cst[sl], G, axis=0)),
        })
    return in_maps


def kernel(pred_rots, pred_P0, pred_Q0, gt_occmask, roi_extent, pred_transes):
    global LAST
    R = np.asarray(pred_rots, np.float32)
    P0 = np.asarray(pred_P0, np.float32)
    Q0 = np.asarray(pred_Q0, np.float32)
    M = np.asarray(gt_occmask, np.float32)
    E = np.asarray(roi_extent, np.float32)
    T = np.asarray(pred_transes, np.float32)

    nc = get_nc()
    cst, sabs = host_constants(R, T, E)
    in_maps = make_in_maps(P0, Q0, M, cst)
    trace = os.environ.get("KERNEL_TRACE", "0") == "1"
    LAST = run_bass_kernel_spmd(nc, in_maps, core_ids=list(range(NCORES)),
                                trace=trace)
    S_a = np.zeros(3, np.float64)
    for k, r in enumerate(LAST.results):
        o = r["out"].astype(np.float64)          # [128, 3]
        st = o.reshape(BPC, G, 3).sum(axis=1)    # [BPC, 3] per-batch
        S_a += (st * sabs[k * BPC:(k + 1) * BPC]).sum(axis=0)
    M_a = M.sum(axis=(0, 2, 3)).astype(np.float64)
    loss = sum(0.0 if M_a[a] < 3 * B else S_a[a] for a in range(3))
    total = max(M_a.sum(), 1.0)
    return np.asarray(np.float32(loss / total))


# revision 4
# speedup vs baseline: 1.1651x; 1.1651x over previous
"""Trainium2 Bass kernel for nn_CT_loss (data-parallel over batch, 8 cores).

Math (R is a general 3x3 matrix, not orthogonal):
  u   = A P0 + b0          A = R diag(e), b0 = t - 0.5 R e      (per batch)
  c   = G P0 + g0          G = R^T A,     g0 = R^T b0
  v_a = A[:,c1] Qa' + A[:,c2] Qb' + h_a  (Q' = Q-0.5), s = R^T t
  d_a = s_a u - c_a v_a ;  la = sqrt(|d_a|^2 m_a)
  loss = sum_a [sum(m_a) >= 3B] sum(la) / max(sum_a sum(m_a), 1)

Device trick: fold 1/s_a into v's affine coefficients (vt = v/s_a), so
  d~_a = u - c_a vt_a  is scalar-free; host multiplies the per-batch
  partial sums by |s_a| during the gather.

v2: measured rates (DVE TT wide 0.58us/kcol, DVE TS ~0.45, ACT 0.95,
GP TS ~2.5 -- slow!). DVE+ACT carry the kernel; GP soaks a few
off-critical-path ops (tv2 a=1,2 + masks 0,1). No stt (measured 1x).

Layout per core: 8 batches; tiles [128, FD=1024]; partition = b*16+g,
free = 1024 pixels. Per-batch scalars ride as per-partition [128,1]
columns of a constants tile. Free-dim sums via accum_out; host finishes
the 128-row + cross-core reduction (the "gather").
"""
import os
import sys

import numpy as np

for _p in ("/opt/trn_rl_repo",):
    if _p not in sys.path:
        sys.path.insert(0, _p)

import concourse.bass as bass
import concourse.bacc as bacc
import concourse.tile as tile
from concourse import mybir
from concourse.bass_utils import run_bass_kernel_spmd

from ml_dtypes import bfloat16

F32 = mybir.dt.float32
BF16 = mybir.dt.bfloat16
AF = mybir.ActivationFunctionType
OP = mybir.AluOpType

B, HW = 64, 128 * 128
NCORES, BPC, G, FD = 8, 8, 16, 1024
F3 = 3 * FD

# a -> (Acol1, Acol2, qchA, qchB)
QCH = {0: (1, 2, 0, 1), 1: (0, 2, 2, 3), 2: (0, 1, 4, 5)}

# constants tile columns
CA = 0    # A[i*3+j] 9
CB0 = 9   # b0 3
CG = 12   # G[a*3+j] 9
CG0 = 21  # g0 3
CV1 = 24  # alpha~[a*3+i] = A[i,c1]/s~_a 9
CHC = 33  # h~[a*3+i] 9
CV2 = 42  # beta~[a*3+i] = A[i,c2]/s~_a 9
CZ = 51   # 0.0
NCST = 52

_BUILT = None
LAST = None


def _bcast3(ap, n):
    """[128, FD] AP -> [128, n, FD] with step-0 middle dim."""
    return bass.AP(tensor=ap.tensor, offset=ap.offset,
                   ap=[ap.ap[0], [0, n], *ap.ap[1:]])


def _build_nc():
    nc = bacc.Bacc(None)
    p0 = nc.dram_tensor("p0", [BPC, G, 3, FD], BF16, kind="ExternalInput")
    q0 = nc.dram_tensor("q0", [BPC, G, 6, FD], BF16, kind="ExternalInput")
    mk = nc.dram_tensor("mk", [BPC, G, 3, FD], BF16, kind="ExternalInput")
    cst = nc.dram_tensor("cst", [128, NCST], F32, kind="ExternalInput")
    outp = nc.dram_tensor("out", [128, 3], F32, kind="ExternalOutput")

    with tile.TileContext(nc) as tc:
        with tc.tile_pool(name="main", bufs=1) as pool:
            # constants first (tiny, scalar HWDGE ring)
            cst_t = pool.tile([128, NCST], F32, tag="cst")
            nc.scalar.dma_start(cst_t[:], cst[:])

            def cs(j):
                return cst_t[:, j:j + 1]

            zero = cs(CZ)

            # warm the Identity table set early (overlaps the input DMA)
            warm = pool.tile([128, 1], BF16, tag="warm")
            nc.scalar.activation(warm[:], cst_t[:, CZ:CZ + 1], AF.Identity,
                                 bias=zero, scale=cs(CZ))

            # ---- input tiles + DMA (sync ring), critical-path order ----
            p0r = p0[:].rearrange("b g c f -> (b g) c f")
            q0r = q0[:].rearrange("b g c f -> (b g) c f")
            mkr = mk[:].rearrange("b g c f -> (b g) c f")

            p0_t = pool.tile([128, 3, FD], BF16, tag="p0")
            q0_t = pool.tile([128, 6, FD], BF16, tag="q0")
            mk_t = pool.tile([128, 3, FD], BF16, tag="mk")

            nc.sync.dma_start(q0_t[:, 0:2, :], q0r[:, 0:2, :])   # a=0 pair
            nc.sync.dma_start(p0_t[:], p0r[:])
            nc.sync.dma_start(q0_t[:, 2:4, :], q0r[:, 2:4, :])   # a=1 pair
            nc.sync.dma_start(q0_t[:, 4:6, :], q0r[:, 4:6, :])   # a=2 pair
            nc.sync.dma_start(mk_t[:], mkr[:])

            acc = pool.tile([128, 3], F32, tag="acc")

            X = [p0_t[:, j, :] for j in range(3)]
            Q = [q0_t[:, j, :] for j in range(6)]
            MSK = [mk_t[:, a, :] for a in range(3)]

            # ---- tiles ----
            u3 = pool.tile([128, 3, FD], BF16, tag="u3")
            ut2 = pool.tile([128, 3, FD], BF16, tag="ut2")
            ut1 = pool.tile([128, 3, FD], BF16, tag="ut1")
            ut0 = pool.tile([128, 3, FD], BF16, tag="ut0")
            ct2 = pool.tile([128, 3, FD], BF16, tag="ct2")
            ct1 = pool.tile([128, 3, FD], BF16, tag="ct1")
            ct0 = pool.tile([128, 3, FD], BF16, tag="ct0")
            c3 = pool.tile([128, 3, FD], BF16, tag="c3")
            tv1 = [pool.tile([128, 3, FD], BF16, name=f"tv1_{a}", tag=f"tv1_{a}")
                   for a in range(3)]
            tv2 = [pool.tile([128, 3, FD], BF16, name=f"tv2_{a}", tag=f"tv2_{a}")
                   for a in range(3)]
            va = [pool.tile([128, 3, FD], BF16, name=f"va{a}", tag=f"va{a}")
                  for a in range(3)]
            sq = [pool.tile([128, 3, FD], BF16, name=f"sq{a}", tag=f"sq{a}")
                  for a in range(3)]
            wt = [pool.tile([128, FD], BF16, name=f"w{a}", tag=f"w{a}")
                  for a in range(3)]
            la = [pool.tile([128, FD], BF16, name=f"la{a}", tag=f"la{a}")
                  for a in range(3)]

            def act_term(out, src, scj, bij):
                nc.scalar.activation(out, src, AF.Identity,
                                     bias=cs(bij), scale=cs(scj))

            def dve_term(out, src, scj, bij=None):
                if bij is None:
                    nc.vector.tensor_scalar(out, src, cs(scj), None,
                                            op0=OP.mult)
                else:
                    nc.vector.tensor_scalar(out, src, cs(scj), cs(bij),
                                            op0=OP.mult, op1=OP.add)

            def gp_term(out, src, scj, bij):
                nc.gpsimd.tensor_scalar(out, src, cs(scj), cs(bij),
                                        op0=OP.mult, op1=OP.add)

            # ============ issue in dataflow order ============
            # DVE: tv2 a=0 (q01 lands first)
            for i in range(3):
                dve_term(tv2[0][:, i, :], Q[QCH[0][3]], CV2 + i)
            # ACT: tv1 a=0 (biased)
            for i in range(3):
                act_term(tv1[0][:, i, :], Q[QCH[0][2]], CV1 + i, CHC + i)
            # ACT: u t2 (biased); DVE: u t1, t0
            for i in range(3):
                act_term(ut2[:, i, :], X[2], CA + 3 * i + 2, CB0 + i)
            for i in range(3):
                dve_term(ut1[:, i, :], X[1], CA + 3 * i + 1)
            for i in range(3):
                dve_term(ut0[:, i, :], X[0], CA + 3 * i)
            # ACT: c t2 (biased); DVE: c t1, t0
            for aa in range(3):
                act_term(ct2[:, aa, :], X[2], CG + 3 * aa + 2, CG0 + aa)
            for aa in range(3):
                dve_term(ct1[:, aa, :], X[1], CG + 3 * aa + 1)
            for aa in range(3):
                dve_term(ct0[:, aa, :], X[0], CG + 3 * aa)

            # GP: tv2 for a=1 (q23) and a=2 (q45), off critical path
            for a in (1, 2):
                for i in range(3):
                    gp_term(tv2[a][:, i, :], Q[QCH[a][3]], CV2 + 3 * a + i, CZ)

            # ACT: tv1 a=1, a=2
            for a in (1, 2):
                for i in range(3):
                    act_term(tv1[a][:, i, :], Q[QCH[a][2]], CV1 + 3 * a + i,
                             CHC + 3 * a + i)

            # DVE: v0 add, u adds, c adds (wide)
            nc.vector.tensor_add(va[0][:], tv1[0][:], tv2[0][:])
            nc.vector.tensor_add(u3[:], ut2[:], ut1[:])
            nc.vector.tensor_add(u3[:], u3[:], ut0[:])
            nc.vector.tensor_add(c3[:], ct2[:], ct1[:])
            nc.vector.tensor_add(c3[:], c3[:], ct0[:])

            # a=0 chain: cv per-i (probe: no bcast3), d wide
            for i in range(3):
                nc.vector.tensor_mul(va[0][:, i, :], c3[:, 0, :],
                                     va[0][:, i, :])
            nc.vector.tensor_sub(va[0][:], u3[:], va[0][:])

            # a=1 chain (bcast3 control)
            nc.vector.tensor_add(va[1][:], tv1[1][:], tv2[1][:])
            nc.vector.tensor_mul(va[1][:], _bcast3(c3[:, 1, :], 3), va[1][:])
            nc.vector.tensor_sub(va[1][:], u3[:], va[1][:])

            # a=2 chain
            nc.vector.tensor_add(va[2][:], tv1[2][:], tv2[2][:])
            nc.vector.tensor_mul(va[2][:], _bcast3(c3[:, 2, :], 3), va[2][:])
            nc.vector.tensor_sub(va[2][:], u3[:], va[2][:])

            # ACT: dummy square (hide table load in the natural stall), sq
            nc.scalar.activation(warm[:], cst_t[:, CZ:CZ + 1], AF.Square)
            for a in range(3):
                nc.scalar.activation(sq[a][:], va[a][:], AF.Square)

            # DVE: w sums; masks: m0,m1 on GP, m2 on DVE
            for a in range(3):
                nc.vector.tensor_add(wt[a], sq[a][:, 0, :], sq[a][:, 1, :])
                nc.vector.tensor_add(wt[a], wt[a], sq[a][:, 2, :])
            nc.gpsimd.tensor_tensor(wt[0], wt[0], MSK[0], op=OP.mult)
            nc.gpsimd.tensor_tensor(wt[1], wt[1], MSK[1], op=OP.mult)
            nc.vector.tensor_mul(wt[2], wt[2], MSK[2])

            # ACT: sqrt + accumulate
            for a in range(3):
                nc.scalar.activation(la[a], wt[a], AF.Sqrt,
                                     accum_out=acc[:, a:a + 1])

            nc.sync.dma_start(outp[:], acc[:])

    nc.compile()
    return nc


def get_nc():
    global _BUILT
    if _BUILT is None:
        _BUILT = _build_nc()
    return _BUILT


def host_constants(R, T, E):
    """[B, NCST] fp32 constants (fp64 host math) + [B,3] |s| scales."""
    Bn = R.shape[0]
    out = np.zeros((Bn, NCST), np.float64)
    sabs = np.zeros((Bn, 3), np.float64)
    for b in range(Bn):
        Rb = R[b].astype(np.float64)
        tb = T[b].astype(np.float64)
        eb = E[b].astype(np.float64)
        A = Rb * eb[None, :]
        b0 = tb - 0.5 * (Rb @ eb)
        Gm = Rb.T @ A
        g0 = Rb.T @ b0
        s = Rb.T @ tb
        out[b, CA:CA + 9] = A.reshape(-1)
        out[b, CB0:CB0 + 3] = b0
        out[b, CG:CG + 9] = Gm.reshape(-1)
        out[b, CG0:CG0 + 3] = g0
        for a, (c1, c2, _, _) in QCH.items():
            sh = np.sign(s[a]) * max(abs(s[a]), 1e-12) if s[a] != 0 else 1e-12
            sabs[b, a] = abs(s[a])
            h = tb - 0.5 * (A[:, c1] + A[:, c2])
            out[b, CV1 + 3 * a:CV1 + 3 * a + 3] = A[:, c1] / sh
            out[b, CV2 + 3 * a:CV2 + 3 * a + 3] = A[:, c2] / sh
            out[b, CHC + 3 * a:CHC + 3 * a + 3] = h / sh
    return out.astype(np.float32), sabs


def make_in_maps(P0, Q0, M, cst):
    in_maps = []
    for k in range(NCORES):
        sl = slice(k * BPC, (k + 1) * BPC)
        in_maps.append({
            "p0": P0[sl].reshape(BPC, 3, G, FD).transpose(0, 2, 1, 3).astype(bfloat16),
            "q0": Q0[sl].reshape(BPC, 6, G, FD).transpose(0, 2, 1, 3).astype(bfloat16),
            "mk": M[sl].reshape(BPC, 3, G, FD).transpose(0, 2, 1, 3).astype(bfloat16),
            "cst": np.ascontiguousarray(np.repeat(cst[sl], G, axis=0)),
        })
    return in_maps


def kernel(pred_rots, pred_P0, pred_Q0, gt_occmask, roi_extent, pred_transes):
    global LAST
    R = np.asarray(pred_rots, np.float32)
    P0 = np.asarray(pred_P0, np.float32)
    Q0 = np.asarray(pred_Q0, np.float32)
    M = np.asarray(gt_occmask, np.float32)
    E = np.asarray(roi_extent, np.float32)
    T = np.asarray(pred_transes, np.float32)

    nc = get_nc()
    cst, sabs = host_constants(R, T, E)
    in_maps = make_in_maps(P0, Q0, M, cst)
    trace = os.environ.get("KERNEL_TRACE", "0") == "1"
    LAST = run_bass_kernel_spmd(nc, in_maps, core_ids=list(range(NCORES)),
                                trace=trace)
    S_a = np.zeros(3, np.float64)
    for k, r in enumerate(LAST.results):
        o = r["out"].astype(np.float64)          # [128, 3]
        st = o.reshape(BPC, G, 3).sum(axis=1)    # [BPC, 3] per-batch
        S_a += (st * sabs[k * BPC:(k + 1) * BPC]).sum(axis=0)
    M_a = M.sum(axis=(0, 2, 3)).astype(np.float64)
    loss = sum(0.0 if M_a[a] < 3 * B else S_a[a] for a in range(3))
    total = max(M_a.sum(), 1.0)
    return np.asarray(np.float32(loss / total))


# revision 14
# speedup vs baseline: 1.2493x; 1.0723x over previous
"""Trainium2 Bass kernel for nn_CT_loss (data-parallel over batch, 8 cores).

Math (R is a general 3x3 matrix, not orthogonal):
  u   = A P0 + b0          A = R diag(e), b0 = t - 0.5 R e      (per batch)
  c   = G P0 + g0          G = R^T A,     g0 = R^T b0
  v_a = A[:,c1] Qa' + A[:,c2] Qb' + h_a  (Q' = Q-0.5), s = R^T t
  d_a = s_a u - c_a v_a ;  la = sqrt(|d_a|^2 m_a)
  loss = sum_a [sum(m_a) >= 3B] sum(la) / max(sum_a sum(m_a), 1)

Device trick: fold 1/s_a into v's affine coefficients (vt = v/s_a), so
  d~_a = u - c_a vt_a  is scalar-free; host multiplies the per-batch
  partial sums by |s_a| during the gather.

v2: measured rates (DVE TT wide 0.58us/kcol, DVE TS ~0.45, ACT 0.95,
GP TS ~2.5 and contends with DVE via the shared SBUF port pair:
GP ops inflate concurrent DVE TTs ~+0.8us). v3: pure DVE+ACT, wide
consolidated u+c adds, queue order tuned for the a-chain pipeline.

Layout per core: 8 batches; tiles [128, FD=1024]; partition = b*16+g,
free = 1024 pixels. Per-batch scalars ride as per-partition [128,1]
columns of a constants tile. Free-dim sums via accum_out; host finishes
the 128-row + cross-core reduction (the "gather").
"""
import os
import sys

import numpy as np

for _p in ("/opt/trn_rl_repo",):
    if _p not in sys.path:
        sys.path.insert(0, _p)

import concourse.bass as bass
import concourse.bacc as bacc
import concourse.tile as tile
from concourse import mybir
from concourse.bass_utils import run_bass_kernel_spmd

from ml_dtypes import bfloat16

F32 = mybir.dt.float32
BF16 = mybir.dt.bfloat16
AF = mybir.ActivationFunctionType
OP = mybir.AluOpType

B, HW = 64, 128 * 128
NCORES, BPC, G, FD = 8, 8, 16, 1024
F3 = 3 * FD

# a -> (Acol1, Acol2, qchA, qchB)
QCH = {0: (1, 2, 0, 1), 1: (0, 2, 2, 3), 2: (0, 1, 4, 5)}

# constants tile columns
CA = 0    # A[i*3+j] 9
CB0 = 9   # b0 3
CG = 12   # G[a*3+j] 9
CG0 = 21  # g0 3
CV1 = 24  # alpha~[a*3+i] = A[i,c1]/s~_a 9
CHC = 33  # h~[a*3+i] 9
CV2 = 42  # beta~[a*3+i] = A[i,c2]/s~_a 9
CZ = 51   # 0.0
NCST = 52

_BUILT = None
LAST = None


def _bcast3(ap, n):
    """[128, FD] AP -> [128, n, FD] with step-0 middle dim."""
    return bass.AP(tensor=ap.tensor, offset=ap.offset,
                   ap=[ap.ap[0], [0, n], *ap.ap[1:]])


def _build_nc():
    nc = bacc.Bacc(None)
    p0 = nc.dram_tensor("p0", [BPC, G, 3, FD], BF16, kind="ExternalInput")
    q0 = nc.dram_tensor("q0", [BPC, G, 6, FD], BF16, kind="ExternalInput")
    mk = nc.dram_tensor("mk", [BPC, G, 3, FD], BF16, kind="ExternalInput")
    cst = nc.dram_tensor("cst", [128, NCST], F32, kind="ExternalInput")
    outp = nc.dram_tensor("out", [128, 2], F32, kind="ExternalOutput")

    with tile.TileContext(nc) as tc:
        with tc.tile_pool(name="main", bufs=1) as pool:
            # constants first on the scalar HWDGE ring (tiny)
            cst_t = pool.tile([128, NCST], F32, tag="cst")
            nc.scalar.dma_start(cst_t[:], cst[:])

            def cs(j):
                return cst_t[:, j:j + 1]

            zero = cs(CZ)

            # warm BOTH act table sets off a memset tile (no cst dep)
            warm = pool.tile([128, 1], BF16, tag="warm")
            nc.vector.memset(warm[:], 0.0)
            nc.scalar.activation(warm[:], warm[:], AF.Sqrt)
            nc.scalar.activation(warm[:], warm[:], AF.Identity,
                                 bias=0.0, scale=1.0)

            # ---- input tiles + DMA, two rings, need-ordered ----
            p0r = p0[:].rearrange("b g c f -> (b g) c f")
            q0r = q0[:].rearrange("b g c f -> (b g) c f")
            mkr = mk[:].rearrange("b g c f -> (b g) c f")

            p0_t = pool.tile([128, 3, FD], BF16, tag="p0")
            q0_t = pool.tile([128, 6, FD], BF16, tag="q0")
            mk_t = pool.tile([128, 3, FD], BF16, tag="mk")

            nc.sync.dma_start(q0_t[:, 1, :], q0r[:, 1, :])       # tv2 a=0 src
            nc.scalar.dma_start(q0_t[:, 0, :], q0r[:, 0, :])     # tv1 a=0 src
            nc.sync.dma_start(p0_t[:], p0r[:])
            nc.scalar.dma_start(q0_t[:, 2:4, :], q0r[:, 2:4, :])  # a=1 pair
            nc.sync.dma_start(q0_t[:, 4:6, :], q0r[:, 4:6, :])    # a=2 pair
            nc.scalar.dma_start(mk_t[:], mkr[:])

            acc = pool.tile([128, 2], F32, tag="acc")

            X = [p0_t[:, j, :] for j in range(3)]
            Q = [q0_t[:, j, :] for j in range(6)]

            # ---- tiles ----
            uc3 = pool.tile([128, 6, FD], BF16, tag="uc3")
            uct2 = pool.tile([128, 6, FD], BF16, tag="uct2")
            uct1 = pool.tile([128, 6, FD], BF16, tag="uct1")
            uct0 = pool.tile([128, 6, FD], BF16, tag="uct0")
            tv1 = [pool.tile([128, 3, FD], BF16, name=f"tv1_{a}", tag=f"tv1_{a}")
                   for a in range(3)]
            tv2 = [pool.tile([128, 3, FD], BF16, name=f"tv2_{a}", tag=f"tv2_{a}")
                   for a in range(3)]
            va = [pool.tile([128, 3, FD], BF16, name=f"va{a}", tag=f"va{a}")
                  for a in range(3)]
            # squares land i-major: channel = i*3 + a
            sqa = pool.tile([128, 9, FD], BF16, tag="sqa")
            wt3 = pool.tile([128, 3, FD], BF16, tag="wt3")
            la3 = pool.tile([128, 3, FD], BF16, tag="la3")
            u3 = uc3[:, 0:3, :]
            c3 = [uc3[:, 3 + a, :] for a in range(3)]

            def sq_slice(a):
                s = sqa[:, a, :]
                return bass.AP(tensor=s.tensor, offset=s.offset,
                               ap=[s.ap[0], [3 * FD, 3], *s.ap[1:]])

            def act_term(out, src, scj, bij):
                nc.scalar.activation(out, src, AF.Identity,
                                     bias=cs(bij), scale=cs(scj))

            def dve_term(out, src, scj, bij=None):
                if bij is None:
                    nc.vector.tensor_scalar(out, src, cs(scj), None,
                                            op0=OP.mult)
                else:
                    nc.vector.tensor_scalar(out, src, cs(scj), cs(bij),
                                            op0=OP.mult, op1=OP.add)

            # ============ issue in dataflow order ============
            # DVE TS: tv2 a=0 (q0 ch1 lands first on sync ring)
            for i in range(3):
                dve_term(tv2[0][:, i, :], Q[QCH[0][3]], CV2 + i)
            # ACT: tv1 a=0 (biased; q0 ch0 first on scalar ring)
            for i in range(3):
                act_term(tv1[0][:, i, :], Q[QCH[0][2]], CV1 + i, CHC + i)
            # ACT: c t2 for a=0 first (feeds the narrow c0 chain), then u t2
            act_term(uct2[:, 3, :], X[2], CG + 2, CG0 + 0)
            for i in range(3):
                act_term(uct2[:, i, :], X[2], CA + 3 * i + 2, CB0 + i)

            # DVE TS: u t1/t0 (p0), then narrow c0 t1/t0
            for i in range(3):
                dve_term(uct1[:, i, :], X[1], CA + 3 * i + 1)
            for i in range(3):
                dve_term(uct0[:, i, :], X[0], CA + 3 * i)
            dve_term(uct1[:, 3, :], X[1], CG + 1)
            dve_term(uct0[:, 3, :], X[0], CG + 0)

            # DVE: narrow c0 chain -> v0 -> cv0 asap
            nc.vector.tensor_add(c3[0], uct2[:, 3, :], uct1[:, 3, :])
            nc.vector.tensor_add(c3[0], c3[0], uct0[:, 3, :])
            nc.vector.tensor_add(va[0][:], tv1[0][:], tv2[0][:])
            for h in (HL, HR):
                nc.vector.tensor_mul(va[0][:, :, h], _bcast3h(c3[0], h),
                                     va[0][:, :, h])

            # ACT: tv1 a=1 while DVE finishes u
            for i in range(3):
                act_term(tv1[1][:, i, :], Q[QCH[1][2]], CV1 + 3 + i,
                         CHC + 3 + i)

            # DVE: u adds (wide over the 3 u channels), d0
            nc.vector.tensor_add(u3, uct2[:, 0:3, :], uct1[:, 0:3, :])
            nc.vector.tensor_add(u3, u3, uct0[:, 0:3, :])
            for h in (HL, HR):
                nc.vector.tensor_sub(va[0][:, :, h], u3[:, :, h],
                                     va[0][:, :, h])

            # ACT: c t2 a=1,2; tv1 a=2 ch0; tv2 a=2
            act_term(uct2[:, 4, :], X[2], CG + 5, CG0 + 1)
            act_term(uct2[:, 5, :], X[2], CG + 8, CG0 + 2)
            act_term(tv1[2][:, 0, :], Q[QCH[2][2]], CV1 + 6, CHC + 6)
            for i in range(3):
                act_term(tv2[2][:, i, :], Q[QCH[2][3]], CV2 + 6 + i, CZ)

            # ACT: sq0 (strided write, i-major sq layout)
            nc.scalar.activation(sq_slice(0), va[0][:], AF.Square)

            # DVE: tv2 a=1 TS + tv1 a=2 ch1,2; c1 chain; a=1 chain
            for i in range(3):
                dve_term(tv2[1][:, i, :], Q[QCH[1][3]], CV2 + 3 + i)
            for i in (1, 2):
                dve_term(tv1[2][:, i, :], Q[QCH[2][2]], CV1 + 6 + i,
                         CHC + 6 + i)
            dve_term(uct1[:, 4, :], X[1], CG + 4)
            dve_term(uct0[:, 4, :], X[0], CG + 3)
            nc.vector.tensor_add(c3[1], uct2[:, 4, :], uct1[:, 4, :])
            nc.vector.tensor_add(c3[1], c3[1], uct0[:, 4, :])
            nc.vector.tensor_add(va[1][:], tv1[1][:], tv2[1][:])
            for h in (HL, HR):
                nc.vector.tensor_mul(va[1][:, :, h], _bcast3h(c3[1], h),
                                     va[1][:, :, h])
                nc.vector.tensor_sub(va[1][:, :, h], u3[:, :, h],
                                     va[1][:, :, h])

            # ACT: sq1
            nc.scalar.activation(sq_slice(1), va[1][:], AF.Square)

            # DVE: c2 chain; a=2 chain
            dve_term(uct1[:, 5, :], X[1], CG + 7)
            dve_term(uct0[:, 5, :], X[0], CG + 6)
            nc.vector.tensor_add(c3[2], uct2[:, 5, :], uct1[:, 5, :])
            nc.vector.tensor_add(c3[2], c3[2], uct0[:, 5, :])
            nc.vector.tensor_add(va[2][:], tv1[2][:], tv2[2][:])
            for h in (HL, HR):
                nc.vector.tensor_mul(va[2][:, :, h], _bcast3h(c3[2], h),
                                     va[2][:, :, h])
                nc.vector.tensor_sub(va[2][:, :, h], u3[:, :, h],
                                     va[2][:, :, h])

            # ACT: sq2
            nc.scalar.activation(sq_slice(2), va[2][:], AF.Square)

            # DVE tails (mask pre-scaled by s^2 * gate on host)
            def w_tail(a):
                nc.vector.tensor_add(wt3[:, a, :], sqa[:, a, :],
                                     sqa[:, 3 + a, :])
                nc.vector.tensor_add(wt3[:, a, :], wt3[:, a, :],
                                     sqa[:, 6 + a, :])
                nc.vector.tensor_mul(wt3[:, a, :], wt3[:, a, :],
                                     mk_t[:, a, :])

            w_tail(0)
            w_tail(1)
            w_tail(2)

            # ACT: sqrt + accumulate (a=0,1 paired; then a=2)
            nc.scalar.activation(la3[:, 0:2, :], wt3[:, 0:2, :], AF.Sqrt,
                                 accum_out=acc[:, 0:1])
            nc.scalar.activation(la3[:, 2, :], wt3[:, 2, :], AF.Sqrt,
                                 accum_out=acc[:, 1:2])

            nc.scalar.dma_start(outp[:], acc[:])

    nc.compile()
    return nc


def get_nc():
    global _BUILT
    if _BUILT is None:
        _BUILT = _build_nc()
    return _BUILT


def host_constants(R, T, E):
    """[B, NCST] fp32 constants (fp64 host math) + [B,3] |s| scales."""
    Bn = R.shape[0]
    out = np.zeros((Bn, NCST), np.float64)
    sabs = np.zeros((Bn, 3), np.float64)
    for b in range(Bn):
        Rb = R[b].astype(np.float64)
        tb = T[b].astype(np.float64)
        eb = E[b].astype(np.float64)
        A = Rb * eb[None, :]
        b0 = tb - 0.5 * (Rb @ eb)
        Gm = Rb.T @ A
        g0 = Rb.T @ b0
        s = Rb.T @ tb
        out[b, CA:CA + 9] = A.reshape(-1)
        out[b, CB0:CB0 + 3] = b0
        out[b, CG:CG + 9] = Gm.reshape(-1)
        out[b, CG0:CG0 + 3] = g0
        for a, (c1, c2, _, _) in QCH.items():
            sh = np.sign(s[a]) * max(abs(s[a]), 1e-12) if s[a] != 0 else 1e-12
            sabs[b, a] = abs(s[a])
            h = tb - 0.5 * (A[:, c1] + A[:, c2])
            out[b, CV1 + 3 * a:CV1 + 3 * a + 3] = A[:, c1] / sh
            out[b, CV2 + 3 * a:CV2 + 3 * a + 3] = A[:, c2] / sh
            out[b, CHC + 3 * a:CHC + 3 * a + 3] = h / sh
    return out.astype(np.float32), sabs


def make_in_maps(P0, Q0, M, cst):
    in_maps = []
    for k in range(NCORES):
        sl = slice(k * BPC, (k + 1) * BPC)
        in_maps.append({
            "p0": P0[sl].reshape(BPC, 3, G, FD).transpose(0, 2, 1, 3).astype(bfloat16),
            "q0": Q0[sl].reshape(BPC, 6, G, FD).transpose(0, 2, 1, 3).astype(bfloat16),
            "mk": M[sl].reshape(BPC, 3, G, FD).transpose(0, 2, 1, 3).astype(bfloat16),
            "cst": np.ascontiguousarray(np.repeat(cst[sl], G, axis=0)),
        })
    return in_maps


def kernel(pred_rots, pred_P0, pred_Q0, gt_occmask, roi_extent, pred_transes):
    global LAST
    R = np.asarray(pred_rots, np.float32)
    P0 = np.asarray(pred_P0, np.float32)
    Q0 = np.asarray(pred_Q0, np.float32)
    M = np.asarray(gt_occmask, np.float32)
    E = np.asarray(roi_extent, np.float32)
    T = np.asarray(pred_transes, np.float32)

    nc = get_nc()
    cst, sabs = host_constants(R, T, E)
    # fold s_a^2 and the per-a gate into the mask (device then computes
    # sqrt(|d~|^2 * m * s^2 * gate) = gate * |s| * la, one accumulator)
    M_a = M.sum(axis=(0, 2, 3)).astype(np.float64)
    gate = (M_a >= 3 * B).astype(np.float64)
    s2g = (sabs.astype(np.float64) ** 2) * gate[None, :]     # [B,3]
    Ms = (M.astype(np.float64) * s2g[:, :, None, None]).astype(np.float32)
    in_maps = make_in_maps(P0, Q0, Ms, cst)
    trace = os.environ.get("KERNEL_TRACE", "0") == "1"
    LAST = run_bass_kernel_spmd(nc, in_maps, core_ids=list(range(NCORES)),
                                trace=trace)
    S = 0.0
    for r in LAST.results:
        S += float(r["out"].astype(np.float64).sum())
    total = max(M_a.sum(), 1.0)
    return np.asarray(np.float32(S / total))
